# Initial kernel scaffold
#
"""Continuous Game-of-Life Trainium2 kernel (v7: plain fp32 input + DVE cast).

Reference computation (per batch image, cyclic 3x3 stencil):
    around = 8-neighbor sum of x (torus wrap)
    survive = sigmoid(10(around-1.5)) * sigmoid(10(3.5-around))
    birth   = sigmoid(10(around-2.5)) * sigmoid(10(3.5-around))
    out     = x*survive + (1-x)*birth

Algebraic simplification (BETA=10 transitions are >= 1.0 apart):
    s_c := sigmoid(10*around - 10*c)
    out ~= x*(s1.5 - s2.5) + (s2.5 - s3.5)    (max abs err 4.5e-5)

Input pipeline (the v1..v5 lesson):
  - fp32->fp16 SWDGE convert-DMA sustains ~214 GB/s aggregate ONLY as a
    clean stream of large transfers; interleaving the 1-row top-halo
    convert-DMA between the big ones collapsed completions to ~4 strips
    per 42 us (measured), which made the whole kernel input-bound.
  - v6 therefore issues exactly ONE large convert-DMA per strip (cells +
    bottom halo, contiguous rows), and fills the top-halo partition with
    a tiny SBUF->SBUF copy from the PREVIOUS strip's tile (its partition
    125 holds row r0-1), issued on the scalar HWDGE ring so it neither
    pollutes the SWDGE stream nor queues behind output DMAs.
  - Only the first strip of each image needs a 1-row DRAM convert-DMA
    for the wrap row (2 per core), plus one for the last strip's bottom
    wrap: rare enough not to disturb the stream.

Per-strip engines:
  - TensorE: 8-neighbor sum via banded fp16 matmuls accumulated in
    PSUM, grouped by stationary operand (m0 center, then m1 shifted).
  - ScalarE: three sigmoids straight out of PSUM (scale/bias fused).
  - VectorE: double-width fp16 sub + mul + add (2x_1P).
  - DMA out: fp16 via sync HWDGE (host upcasts to fp32).

Engine budget per core (2 images, 34 strips):
  DMA  34 MB convert-in (~214 GB/s) + 17 MB out  ~199 us <- bottleneck
  ACT  3 sigmoid passes [126,2048]               ~189 us
  DVE  sub(2W)+mul+add fp16                      ~161 us
  PE   14 matmuls + ldweights per strip          ~150 us

Sharding: pure data-parallel over batch: 16 images -> 8 cores x 2 images.
The torus wrap is per-image so there is no cross-core halo at all.
"""

import numpy as np

B, H, W = 16, 2048, 2048
N_CORES = 8
B_PER = B // N_CORES  # 2 images per core
STRIDE = 126  # output rows per strip (128 input rows incl. 1-row halos)
N_STRIPS = (H + STRIDE - 1) // STRIDE  # 17
NBANKS = W // 512  # PSUM banks per strip

_cached_nc = None


def _band_matrices(m, dtype=np.float16):
    """[m+2, m] stationary operands for the vertical taps.

    Tile layout: partitions 0..m-1 hold image rows r0..r0+m-1 (the cells),
    partition m holds the bottom halo row r0+m, partition m+1 holds the top
    halo row r0-1.  For output row p the vertical neighbors are partitions
    p-1 (or m+1 when p==0) and p+1.

    m0[k, p] = 1 for the two vertical neighbors (no center),
    m1[k, p] = 1 for the full 3-tap (used on the column-shifted views).
    """
    m0 = np.zeros((m + 2, m), dtype)
    m1 = np.zeros((m + 2, m), dtype)
    for p in range(m):
        up = m + 1 if p == 0 else p - 1
        m0[up, p] = 1.0
        m0[p + 1, p] = 1.0
        m1[up, p] = 1.0
        m1[p, p] = 1.0
        m1[p + 1, p] = 1.0
    return m0, m1


def _build(b_per=B_PER, h=H, w=W, stride=STRIDE):
    global _cached_nc
    if _cached_nc is not None and (b_per, h, w, stride) == (B_PER, H, W, STRIDE):
        return _cached_nc

    import concourse.mybir as mybir
    from concourse.bacc import Bacc
    from concourse.tile import TileContext

    B_PER_, H_, W_, STRIDE_ = b_per, h, w, stride
    N_STRIPS_ = (H_ + STRIDE_ - 1) // STRIDE_
    NBANKS_ = W_ // 512
    KROWS = STRIDE_ + 2  # input rows per full strip

    f32 = mybir.dt.float32
    f16 = mybir.dt.float16
    Sig = mybir.ActivationFunctionType.Sigmoid

    nc = Bacc(trn_type="TRN2")
    x_d = nc.dram_tensor("x", [B_PER_, H_, W_], f32, kind="ExternalInput")
    y_d = nc.dram_tensor("y", [B_PER_, H_, W_], f16, kind="ExternalOutput")

    consts = {}
    for m in sorted({STRIDE_, H_ - STRIDE_ * (N_STRIPS_ - 1)}):
        m0_np, m1_np = _band_matrices(m)
        consts[m] = (
            nc.inline_tensor(m0_np, f"m0_const_{m}"),
            nc.inline_tensor(m1_np, f"m1_const_{m}"),
        )

    with TileContext(nc) as tc:
        with (
            tc.tile_pool(name="wpool", bufs=1) as wpool,
            tc.tile_pool(name="fpool", bufs=6) as fpool,
            tc.tile_pool(name="xpool", bufs=4) as xpool,
            tc.tile_pool(name="spool", bufs=4) as spool,
            tc.tile_pool(name="dpool", bufs=4) as dpool,
            tc.tile_pool(name="mpool", bufs=3) as mpool,
            tc.tile_pool(name="opool", bufs=5) as opool,
            tc.tile_pool(name="ppool", bufs=2, space="PSUM") as ppool,
        ):
            bands = {}
            for m, (m0_d, m1_d) in consts.items():
                m0 = wpool.tile([m + 2, m], f16, name=f"m0_{m}")
                m1 = wpool.tile([m + 2, m], f16, name=f"m1_{m}")
                nc.sync.dma_start(out=m0[:], in_=m0_d[:])
                nc.sync.dma_start(out=m1[:], in_=m1_d[:])
                bands[m] = (m0, m1)

            # activation biases must be [128,1] APs, not immediates
            b15 = wpool.tile([128, 1], f32)
            b25 = wpool.tile([128, 1], f32)
            b35 = wpool.tile([128, 1], f32)
            nc.vector.memset(b15[:], -15.0)
            nc.vector.memset(b25[:], -25.0)
            nc.vector.memset(b35[:], -35.0)

            for b in range(B_PER_):
                for t in range(N_STRIPS_):
                    r0 = t * STRIDE_  # first output row
                    M = min(STRIDE_, H_ - r0)  # output rows this strip
                    k = M + 2  # partitions used (cells + 2 halos)
                    m0, m1 = bands[M]

                    # fp32 tile, partitions 0..M-1 = cells (rows r0..),
                    # partition M = bottom halo, M+1 = top halo.  Plain
                    # SWDGE DMAs (~420 GB/s); the fp32->fp16 convert-DMA
                    # measured at most half that and collapsed entirely
                    # when mixed with the output stream.
                    xf = fpool.tile([KROWS, W_], f32, tag="xf")
                    if r0 + M < H_:
                        # cells + bottom halo contiguous
                        nc.gpsimd.dma_start(
                            out=xf[0 : M + 1, :], in_=x_d[b, r0 : r0 + M + 1, :]
                        )
                    else:
                        # last strip: bottom halo wraps to row 0
                        nc.gpsimd.dma_start(out=xf[0:M, :], in_=x_d[b, r0:H_, :])
                        nc.gpsimd.dma_start(out=xf[M : M + 1, :], in_=x_d[b, 0:1, :])
                    rtop = (r0 - 1) % H_
                    nc.gpsimd.dma_start(
                        out=xf[M + 1 : M + 2, :], in_=x_d[b, rtop : rtop + 1, :]
                    )
                    # fp32 -> fp16 on DVE (tensor_copy, 2x_2P single-src)
                    xt = xpool.tile([KROWS, W_], f16, tag="xt")
                    nc.vector.tensor_copy(out=xt[:k, :], in_=xf[:k, :])

                    ps = ppool.tile([STRIDE_, W_], f32, tag="ps")
                    m0s = m0[:k, :M]
                    m1s = m1[:k, :M]

                    # Pre-touch: a 1x1 matmul absorbs the PSUM-release wait
                    # (Matmult carries at most ONE sync wait; without this,
                    # Bacc's wait-merging couples strip t to strip t-1's
                    # activations and serializes PE behind ACT).
                    nc.tensor.matmul(
                        ps[:1, 0:1], b15[:1, :1], b15[:1, :1],
                        start=True, stop=True,
                    )

                    # around = 8-neighbor sum accumulated in PSUM, grouped
                    # by stationary operand to minimize weight switches.
                    # m0 group: center column, vertical neighbors only.
                    for nb in range(NBANKS_):
                        c0 = nb * 512
                        nc.tensor.matmul(
                            ps[:M, c0 : c0 + 512], m0s, xt[:k, c0 : c0 + 512],
                            start=True, stop=False,
                        )
                    # m1 group, left-neighbor column: out col j += band @ x col j-1
                    for nb in range(NBANKS_):
                        c0 = nb * 512
                        c1 = c0 + 512
                        if nb == 0:
                            nc.tensor.matmul(
                                ps[:M, 1:512], m1s, xt[:k, 0:511],
                                start=False, stop=False,
                            )
                            nc.tensor.matmul(
                                ps[:M, 0:1], m1s, xt[:k, W_ - 1 : W_],
                                start=False, stop=False,
                            )
                        else:
                            nc.tensor.matmul(
                                ps[:M, c0:c1], m1s, xt[:k, c0 - 1 : c1 - 1],
                                start=False, stop=False,
                            )
                    # m1 group, right-neighbor column: out col j += band @ x col j+1
                    for nb in range(NBANKS_):
                        c0 = nb * 512
                        c1 = c0 + 512
                        if nb == NBANKS_ - 1:
                            nc.tensor.matmul(
                                ps[:M, c0 : W_ - 1], m1s, xt[:k, c0 + 1 : W_],
                                start=False, stop=False,
                            )
                            nc.tensor.matmul(
                                ps[:M, W_ - 1 : W_], m1s, xt[:k, 0:1],
                                start=False, stop=True,
                            )
                        else:
                            nc.tensor.matmul(
                                ps[:M, c0:c1], m1s, xt[:k, c0 + 1 : c1 + 1],
                                start=False, stop=True,
                            )

                    # one contiguous tile [s15 | s25 | s35] so a single
                    # double-width DVE sub computes d=s15-s25 and e=s25-s35
                    # via overlapping slices
                    sall = spool.tile([STRIDE_, 3 * W_], f16, tag="sall")
                    nc.scalar.activation(sall[:M, 0:W_], ps[:M], Sig, bias=b15[:M], scale=10.0)
                    nc.scalar.activation(sall[:M, W_ : 2 * W_], ps[:M], Sig, bias=b25[:M], scale=10.0)
                    nc.scalar.activation(sall[:M, 2 * W_ : 3 * W_], ps[:M], Sig, bias=b35[:M], scale=10.0)

                    de = dpool.tile([STRIDE_, 2 * W_], f16, tag="de")
                    nc.vector.tensor_sub(
                        out=de[:M], in0=sall[:M, 0 : 2 * W_], in1=sall[:M, W_ : 3 * W_]
                    )
                    m_t = mpool.tile([STRIDE_, W_], f16, tag="m")
                    o = opool.tile([STRIDE_, W_], f16, tag="o")
                    nc.vector.tensor_mul(out=m_t[:M], in0=xt[:M, :], in1=de[:M, 0:W_])
                    nc.vector.tensor_add(out=o[:M], in0=m_t[:M], in1=de[:M, W_ : 2 * W_])
                    nc.sync.dma_start(out=y_d[b, r0 : r0 + M, :], in_=o[:M])

    nc.compile()
    if (b_per, h, w, stride) == (B_PER, H, W, STRIDE):
        _cached_nc = nc
    return nc


def run(x, trace=False):
    """Run the SPMD kernel on 8 cores. Returns (out_fp32, BassKernelResults)."""
    from concourse.bass_utils import run_bass_kernel_spmd

    nc = _build()
    x = np.asarray(x, dtype=np.float32)
    assert x.shape == (B, H, W), x.shape
    in_maps = [{"x": x[B_PER * c : B_PER * (c + 1)]} for c in range(N_CORES)]
    res = run_bass_kernel_spmd(nc, in_maps, core_ids=list(range(N_CORES)), trace=trace)
    out = np.concatenate(
        [res.results[c]["y"].astype(np.float32) for c in range(N_CORES)], axis=0
    )
    return out, res


def kernel(x):
    out, _ = run(x, trace=False)
    return out



# revision 1
# speedup vs baseline: 2.8760x; 2.8760x over previous
"""Continuous Game-of-Life Trainium2 kernel (v7: plain fp32 input + DVE cast).

Reference computation (per batch image, cyclic 3x3 stencil):
    around = 8-neighbor sum of x (torus wrap)
    survive = sigmoid(10(around-1.5)) * sigmoid(10(3.5-around))
    birth   = sigmoid(10(around-2.5)) * sigmoid(10(3.5-around))
    out     = x*survive + (1-x)*birth

Algebraic simplification (BETA=10 transitions are >= 1.0 apart):
    s_c := sigmoid(10*around - 10*c)
    out ~= x*(s1.5 - s2.5) + (s2.5 - s3.5)    (max abs err 4.5e-5)

Input pipeline (the v1..v5 lesson):
  - fp32->fp16 SWDGE convert-DMA sustains ~214 GB/s aggregate ONLY as a
    clean stream of large transfers; interleaving the 1-row top-halo
    convert-DMA between the big ones collapsed completions to ~4 strips
    per 42 us (measured), which made the whole kernel input-bound.
  - v6 therefore issues exactly ONE large convert-DMA per strip (cells +
    bottom halo, contiguous rows), and fills the top-halo partition with
    a tiny SBUF->SBUF copy from the PREVIOUS strip's tile (its partition
    125 holds row r0-1), issued on the scalar HWDGE ring so it neither
    pollutes the SWDGE stream nor queues behind output DMAs.
  - Only the first strip of each image needs a 1-row DRAM convert-DMA
    for the wrap row (2 per core), plus one for the last strip's bottom
    wrap: rare enough not to disturb the stream.

Per-strip engines:
  - TensorE: 8-neighbor sum via banded fp16 matmuls accumulated in
    PSUM, grouped by stationary operand (m0 center, then m1 shifted).
  - ScalarE: three sigmoids straight out of PSUM (scale/bias fused).
  - VectorE: double-width fp16 sub + mul + add (2x_1P).
  - DMA out: fp16 via sync HWDGE (host upcasts to fp32).

Engine budget per core (2 images, 34 strips):
  DMA  34 MB convert-in (~214 GB/s) + 17 MB out  ~199 us <- bottleneck
  ACT  3 sigmoid passes [126,2048]               ~189 us
  DVE  sub(2W)+mul+add fp16                      ~161 us
  PE   14 matmuls + ldweights per strip          ~150 us

Sharding: pure data-parallel over batch: 16 images -> 8 cores x 2 images.
The torus wrap is per-image so there is no cross-core halo at all.
"""

import numpy as np

B, H, W = 16, 2048, 2048
N_CORES = 8
B_PER = B // N_CORES  # 2 images per core
STRIDE = 126  # output rows per strip (128 input rows incl. 1-row halos)
N_STRIPS = (H + STRIDE - 1) // STRIDE  # 17
NBANKS = W // 512  # PSUM banks per strip

_cached_nc = None


def _band_matrices(m, dtype=np.float16):
    """[m+2, m] stationary operands for the vertical taps.

    Tile layout: partitions 0..m-1 hold image rows r0..r0+m-1 (the cells),
    partition m holds the bottom halo row r0+m, partition m+1 holds the top
    halo row r0-1.  For output row p the vertical neighbors are partitions
    p-1 (or m+1 when p==0) and p+1.

    m0[k, p] = 1 for the two vertical neighbors (no center),
    m1[k, p] = 1 for the full 3-tap (used on the column-shifted views).
    """
    m0 = np.zeros((m + 2, m), dtype)
    m1 = np.zeros((m + 2, m), dtype)
    for p in range(m):
        up = m + 1 if p == 0 else p - 1
        m0[up, p] = 1.0
        m0[p + 1, p] = 1.0
        m1[up, p] = 1.0
        m1[p, p] = 1.0
        m1[p + 1, p] = 1.0
    return m0, m1


def _build(b_per=B_PER, h=H, w=W, stride=STRIDE):
    global _cached_nc
    if _cached_nc is not None and (b_per, h, w, stride) == (B_PER, H, W, STRIDE):
        return _cached_nc

    import concourse.mybir as mybir
    from concourse.bacc import Bacc
    from concourse.tile import TileContext

    B_PER_, H_, W_, STRIDE_ = b_per, h, w, stride
    N_STRIPS_ = (H_ + STRIDE_ - 1) // STRIDE_
    NBANKS_ = W_ // 512
    KROWS = STRIDE_ + 2  # input rows per full strip

    f32 = mybir.dt.float32
    f16 = mybir.dt.float16
    Sig = mybir.ActivationFunctionType.Sigmoid

    nc = Bacc(trn_type="TRN2")
    x_d = nc.dram_tensor("x", [B_PER_, H_, W_], f32, kind="ExternalInput")
    y_d = nc.dram_tensor("y", [B_PER_, H_, W_], f16, kind="ExternalOutput")

    consts = {}
    for m in sorted({STRIDE_, H_ - STRIDE_ * (N_STRIPS_ - 1)}):
        m0_np, m1_np = _band_matrices(m)
        consts[m] = (
            nc.inline_tensor(m0_np, f"m0_const_{m}"),
            nc.inline_tensor(m1_np, f"m1_const_{m}"),
        )

    with TileContext(nc) as tc:
        with (
            tc.tile_pool(name="wpool", bufs=1) as wpool,
            tc.tile_pool(name="fpool", bufs=6) as fpool,
            tc.tile_pool(name="xpool", bufs=4) as xpool,
            tc.tile_pool(name="spool", bufs=4) as spool,
            tc.tile_pool(name="dpool", bufs=4) as dpool,
            tc.tile_pool(name="mpool", bufs=3) as mpool,
            tc.tile_pool(name="opool", bufs=5) as opool,
            tc.tile_pool(name="ppool", bufs=2, space="PSUM") as ppool,
        ):
            bands = {}
            for m, (m0_d, m1_d) in consts.items():
                m0 = wpool.tile([m + 2, m], f16, name=f"m0_{m}")
                m1 = wpool.tile([m + 2, m], f16, name=f"m1_{m}")
                nc.sync.dma_start(out=m0[:], in_=m0_d[:])
                nc.sync.dma_start(out=m1[:], in_=m1_d[:])
                bands[m] = (m0, m1)

            # activation biases must be [128,1] APs, not immediates
            b15 = wpool.tile([128, 1], f32)
            b25 = wpool.tile([128, 1], f32)
            b35 = wpool.tile([128, 1], f32)
            nc.vector.memset(b15[:], -15.0)
            nc.vector.memset(b25[:], -25.0)
            nc.vector.memset(b35[:], -35.0)

            for b in range(B_PER_):
                for t in range(N_STRIPS_):
                    r0 = t * STRIDE_  # first output row
                    M = min(STRIDE_, H_ - r0)  # output rows this strip
                    k = M + 2  # partitions used (cells + 2 halos)
                    m0, m1 = bands[M]

                    # fp32 tile, partitions 0..M-1 = cells (rows r0..),
                    # partition M = bottom halo, M+1 = top halo.  Plain
                    # SWDGE DMAs (~420 GB/s); the fp32->fp16 convert-DMA
                    # measured at most half that and collapsed entirely
                    # when mixed with the output stream.
                    xf = fpool.tile([KROWS, W_], f32, tag="xf")
                    if r0 + M < H_:
                        # cells + bottom halo contiguous
                        nc.gpsimd.dma_start(
                            out=xf[0 : M + 1, :], in_=x_d[b, r0 : r0 + M + 1, :]
                        )
                    else:
                        # last strip: bottom halo wraps to row 0
                        nc.gpsimd.dma_start(out=xf[0:M, :], in_=x_d[b, r0:H_, :])
                        nc.gpsimd.dma_start(out=xf[M : M + 1, :], in_=x_d[b, 0:1, :])
                    rtop = (r0 - 1) % H_
                    nc.gpsimd.dma_start(
                        out=xf[M + 1 : M + 2, :], in_=x_d[b, rtop : rtop + 1, :]
                    )
                    # fp32 -> fp16 on DVE (tensor_copy, 2x_2P single-src)
                    xt = xpool.tile([KROWS, W_], f16, tag="xt")
                    nc.vector.tensor_copy(out=xt[:k, :], in_=xf[:k, :])

                    ps = ppool.tile([STRIDE_, W_], f32, tag="ps")
                    m0s = m0[:k, :M]
                    m1s = m1[:k, :M]

                    # Pre-touch: a 1x1 matmul absorbs the PSUM-release wait
                    # (Matmult carries at most ONE sync wait; without this,
                    # Bacc's wait-merging couples strip t to strip t-1's
                    # activations and serializes PE behind ACT).
                    nc.tensor.matmul(
                        ps[:1, 0:1], b15[:1, :1], b15[:1, :1],
                        start=True, stop=True,
                    )

                    # around = 8-neighbor sum accumulated in PSUM, grouped
                    # by stationary operand to minimize weight switches.
                    # m0 group: center column, vertical neighbors only.
                    for nb in range(NBANKS_):
                        c0 = nb * 512
                        nc.tensor.matmul(
                            ps[:M, c0 : c0 + 512], m0s, xt[:k, c0 : c0 + 512],
                            start=True, stop=False,
                        )
                    # m1 group, left-neighbor column: out col j += band @ x col j-1
                    for nb in range(NBANKS_):
                        c0 = nb * 512
                        c1 = c0 + 512
                        if nb == 0:
                            nc.tensor.matmul(
                                ps[:M, 1:512], m1s, xt[:k, 0:511],
                                start=False, stop=False,
                            )
                            nc.tensor.matmul(
                                ps[:M, 0:1], m1s, xt[:k, W_ - 1 : W_],
                                start=False, stop=False,
                            )
                        else:
                            nc.tensor.matmul(
                                ps[:M, c0:c1], m1s, xt[:k, c0 - 1 : c1 - 1],
                                start=False, stop=False,
                            )
                    # m1 group, right-neighbor column: out col j += band @ x col j+1
                    for nb in range(NBANKS_):
                        c0 = nb * 512
                        c1 = c0 + 512
                        if nb == NBANKS_ - 1:
                            nc.tensor.matmul(
                                ps[:M, c0 : W_ - 1], m1s, xt[:k, c0 + 1 : W_],
                                start=False, stop=False,
                            )
                            nc.tensor.matmul(
                                ps[:M, W_ - 1 : W_], m1s, xt[:k, 0:1],
                                start=False, stop=True,
                            )
                        else:
                            nc.tensor.matmul(
                                ps[:M, c0:c1], m1s, xt[:k, c0 + 1 : c1 + 1],
                                start=False, stop=True,
                            )

                    # one contiguous tile [s15 | s25 | s35] so a single
                    # double-width DVE sub computes d=s15-s25 and e=s25-s35
                    # via overlapping slices
                    sall = spool.tile([STRIDE_, 3 * W_], f16, tag="sall")
                    nc.scalar.activation(sall[:M, 0:W_], ps[:M], Sig, bias=b15[:M], scale=10.0)
                    nc.scalar.activation(sall[:M, W_ : 2 * W_], ps[:M], Sig, bias=b25[:M], scale=10.0)
                    nc.scalar.activation(sall[:M, 2 * W_ : 3 * W_], ps[:M], Sig, bias=b35[:M], scale=10.0)

                    de = dpool.tile([STRIDE_, 2 * W_], f16, tag="de")
                    nc.vector.tensor_sub(
                        out=de[:M], in0=sall[:M, 0 : 2 * W_], in1=sall[:M, W_ : 3 * W_]
                    )
                    m_t = mpool.tile([STRIDE_, W_], f16, tag="m")
                    o = opool.tile([STRIDE_, W_], f16, tag="o")
                    nc.vector.tensor_mul(out=m_t[:M], in0=xt[:M, :], in1=de[:M, 0:W_])
                    nc.vector.tensor_add(out=o[:M], in0=m_t[:M], in1=de[:M, W_ : 2 * W_])
                    nc.sync.dma_start(out=y_d[b, r0 : r0 + M, :], in_=o[:M])

    nc.compile()
    if (b_per, h, w, stride) == (B_PER, H, W, STRIDE):
        _cached_nc = nc
    return nc


def run(x, trace=False):
    """Run the SPMD kernel on 8 cores. Returns (out_fp32, BassKernelResults)."""
    from concourse.bass_utils import run_bass_kernel_spmd

    nc = _build()
    x = np.asarray(x, dtype=np.float32)
    assert x.shape == (B, H, W), x.shape
    in_maps = [{"x": x[B_PER * c : B_PER * (c + 1)]} for c in range(N_CORES)]
    res = run_bass_kernel_spmd(nc, in_maps, core_ids=list(range(N_CORES)), trace=trace)
    out = np.concatenate(
        [res.results[c]["y"].astype(np.float32) for c in range(N_CORES)], axis=0
    )
    return out, res


def kernel(x):
    out, _ = run(x, trace=False)
    return out

